# revision 14
# baseline (speedup 1.0000x reference)
"""Trainium2 Bass kernel for nn_CropperQAT (multi-scale RoIAlign with
fake-quantized rois).

Strategy (pure data-parallel over (roi, scale) jobs, 8 cores):
  * Host replicates the reference roi math bit-exactly, classifies jobs:
      - REG (x-regular, ~94%): the 8 bilinear y-neighbor pairs of a job all
        lie in one 9-row window -> gather the 9 source rows ONCE (9
        descriptors/job instead of 16; the gather is HBM-latency-bound per
        descriptor, not per byte).  y-interp runs on the PE as a per-group
        block-diagonal matmul W[126,112] (14 jobs x 9 rows -> 14 jobs x 8
        out rows), x-interp as ACT copy-scale + DVE stt on shifted slices.
      - IRR (x-irregular): previous path — 2 row gathers per out row,
        ACT+DVE y-interp, dense 9-term x-interp.
  * Features are gathered from an fp16 copy of the channels-last feature
    table (host-converted); interp weights exact in fp16 / fp32.
"""
import os
import sys

sys.path.insert(0, "/opt/trn_rl_repo")

import numpy as np

import concourse.bass as bass
import concourse.bacc as bacc
import concourse.mybir as mybir
from concourse.tile import TileContext
from concourse.bass_utils import run_bass_kernel_spmd

F32 = np.float32
F16 = np.float16
SIZE = 8
STRIDES = (4, 8, 16)
QS = np.float32(0.25)
C = 64
N_CORES = 8
P = 128
RJOBS = 14                   # reg: 14 jobs x 9 window rows = 126 partitions
RPART = RJOBS * 9            # 126
ROUT = RJOBS * SIZE          # 112 output partitions
IJOBS = 16                   # irr: 16 jobs x 8 rows = 128 partitions
IRRW = 76                    # irr weight row: wl, wh, pad, pad, M[9*8]

LAST_RESULTS = None          # BassKernelResults of the most recent run


# ----------------------------------------------------------------------------
# host-side math (bit-exact replication of the jax reference)
# ----------------------------------------------------------------------------

def _fake_quant(x):
    return (np.clip(np.round(x / QS), -32768, 32767) * QS).astype(F32)


def _prep(c, L):
    valid = (c >= -1.0) & (c <= L)
    c = np.maximum(c, F32(0.0))
    low0 = np.floor(c).astype(np.int32)
    hi_edge = low0 >= L - 1
    low = np.where(hi_edge, L - 1, low0).astype(np.int32)
    high = np.where(hi_edge, L - 1, low0 + 1).astype(np.int32)
    c = np.where(hi_edge, F32(L - 1), c).astype(F32)
    frac = (c - low.astype(F32)).astype(F32)
    return low, high, frac, valid


def _scale_tables(pixel, batch_index, stride, H, W, base):
    A = pixel.shape[0]
    st = F32(stride)
    half = F32(SIZE / 2.0)
    centers = (np.arange(SIZE, dtype=F32) + F32(0.5)).astype(F32)

    px = pixel[:, 0].astype(F32)
    py = pixel[:, 1].astype(F32)
    x1 = _fake_quant(np.maximum(px / st - half, F32(0.0)).astype(F32))
    y1 = _fake_quant(np.maximum(py / st - half, F32(0.0)).astype(F32))
    x2 = _fake_quant(np.maximum(px / st + half, F32(0.0)).astype(F32))
    y2 = _fake_quant(np.maximum(py / st + half, F32(0.0)).astype(F32))
    roi_w = np.maximum(x2 - x1, F32(1.0)).astype(F32)
    roi_h = np.maximum(y2 - y1, F32(1.0)).astype(F32)
    y = (y1[:, None] + centers[None, :] * (roi_h / F32(SIZE))[:, None]).astype(F32)
    x = (x1[:, None] + centers[None, :] * (roi_w / F32(SIZE))[:, None]).astype(F32)

    yl, yh, fy, vy = _prep(y, H)
    xl, xh, fx, vx = _prep(x, W)

    b = batch_index.astype(np.int64)
    x0 = np.minimum(xl[:, 0], W - 9).astype(np.int64)   # 9-px window start
    row_lo = base + (b[:, None] * H + yl.astype(np.int64)) * W + x0[:, None]
    row_hi = base + (b[:, None] * H + yh.astype(np.int64)) * W + x0[:, None]

    vyf = vy.astype(F32)
    wl = ((F32(1.0) - fy) * vyf).astype(F32)            # [A, 8]
    wh = (fy * vyf).astype(F32)

    # x-regular classification
    ar = np.arange(SIZE, dtype=np.int32)
    reg = (np.all(xl == xl[:, :1] + ar[None, :], axis=1)
           & np.all(xh == xl + 1, axis=1)
           & np.all(vx, axis=1)
           & np.all(fx == fx[:, :1], axis=1))

    bx0 = (F32(1.0) - fx[:, 0]).astype(F32)
    bx1 = fx[:, 0].astype(F32)

    # 9-row y window (all yl/yh of a job fit in [ys, ys+8]) + y-interp matrix
    ys = np.minimum(yl[:, 0], H - 9).astype(np.int64)
    e_lo = yl.astype(np.int64) - ys[:, None]
    e_hi = yh.astype(np.int64) - ys[:, None]
    assert e_lo.min() >= 0 and e_lo.max() <= 8
    assert e_hi.min() >= 0 and e_hi.max() <= 8
    Wy = np.zeros((A, 9, SIZE), F32)
    aa = np.repeat(np.arange(A), SIZE)
    ii = np.tile(np.arange(SIZE), A)
    np.add.at(Wy, (aa, e_lo.ravel(), ii), wl.ravel())
    np.add.at(Wy, (aa, e_hi.ravel(), ii), wh.ravel())
    row9 = (base + (b[:, None] * H + ys[:, None]
                    + np.arange(9)[None, :]) * W + x0[:, None]).astype(np.int32)

    # dense x matrix for irregular jobs: M[a, e, j]
    e_xlo = (xl.astype(np.int64) - x0[:, None])
    e_xhi = (xh.astype(np.int64) - x0[:, None])
    assert e_xlo.min() >= 0 and e_xlo.max() <= 8
    assert e_xhi.min() >= 0 and e_xhi.max() <= 8
    vxf = vx.astype(F32)
    M = np.zeros((A, 9, SIZE), F32)
    jj = np.tile(np.arange(SIZE), A)
    np.add.at(M, (aa, e_xlo.ravel(), jj), ((F32(1.0) - fx) * vxf).ravel())
    np.add.at(M, (aa, e_xhi.ravel(), jj), (fx * vxf).ravel())

    return dict(row_lo=row_lo, row_hi=row_hi, wl=wl, wh=wh,
                bx0=bx0, bx1=bx1, M=M, reg=reg, Wy=Wy, row9=row9)


def _host_prep(f0, f1, f2, pixel, batch_index):
    A = pixel.shape[0]
    feats = (f0, f1, f2)
    shapes = [(f.shape[2], f.shape[3]) for f in feats]

    cat = np.concatenate([
        np.ascontiguousarray(np.asarray(f, dtype=F32).transpose(0, 2, 3, 1)).reshape(-1, C)
        for f in feats], axis=0).astype(F16)

    tabs = []
    base = 0
    for s, (H, W) in enumerate(shapes):
        tabs.append(_scale_tables(np.asarray(pixel, F32), np.asarray(batch_index),
                                  STRIDES[s], H, W, base))
        base += 4 * H * W

    reg_jobs = np.array([(s, a) for s in range(3)
                         for a in np.nonzero(tabs[s]["reg"])[0]], dtype=np.int64)
    irr_jobs = np.array([(s, a) for s in range(3)
                         for a in np.nonzero(~tabs[s]["reg"])[0]], dtype=np.int64)
    if len(irr_jobs) == 0:
        irr_jobs = reg_jobs[:1].copy()
    core_reg = [reg_jobs[c::N_CORES] for c in range(N_CORES)]
    core_irr = [irr_jobs[c::N_CORES] for c in range(N_CORES)]
    NRG = max(-(-max(len(x) for x in core_reg) // RJOBS), 1)
    NIG = max(-(-max(len(x) for x in core_irr) // IJOBS), 1)

    per_core = []
    for ccc in range(N_CORES):
        rj = core_reg[ccc]
        ij = core_irr[ccc]
        if len(rj) == 0:
            rj = reg_jobs[:1].copy()
        if len(ij) == 0:
            ij = irr_jobs[:1].copy()
        rpad = np.concatenate([rj, np.repeat(rj[-1:], NRG * RJOBS - len(rj), axis=0)])
        ipad = np.concatenate([ij, np.repeat(ij[-1:], NIG * IJOBS - len(ij), axis=0)])

        # ---- REG tables: idx [126, NRG], Wy [126, NRG*112], bx [112, NRG*2]
        r9 = np.stack([tabs[s]["row9"][a] for s, a in rpad])          # [NRG*14, 9]
        wy = np.stack([tabs[s]["Wy"][a] for s, a in rpad])            # [NRG*14, 9, 8]
        bx0 = np.stack([tabs[s]["bx0"][a] for s, a in rpad])          # [NRG*14]
        bx1 = np.stack([tabs[s]["bx1"][a] for s, a in rpad])

        ridx = (r9.reshape(NRG, RJOBS, 9).transpose(1, 2, 0)
                .reshape(RPART, NRG)).astype(np.int32)
        wmat = np.zeros((NRG, RPART, ROUT), F16)
        wy_g = wy.reshape(NRG, RJOBS, 9, SIZE)
        for j in range(RJOBS):
            wmat[:, 9 * j: 9 * j + 9, SIZE * j: SIZE * j + SIZE] = wy_g[:, j]
        wmat = np.ascontiguousarray(
            wmat.transpose(1, 0, 2).reshape(RPART, NRG * ROUT), F16)
        rbx = np.zeros((NRG, ROUT, 2), F32)
        rbx[:, :, 0] = np.repeat(bx0.reshape(NRG, RJOBS), SIZE, axis=1)
        rbx[:, :, 1] = np.repeat(bx1.reshape(NRG, RJOBS), SIZE, axis=1)
        rbx = np.ascontiguousarray(
            rbx.transpose(1, 0, 2).reshape(ROUT, NRG * 2), F32)

        # ---- IRR tables (identical to previous kernel)
        def job_rows(jobs):
            rl = np.stack([tabs[s]["row_lo"][a] for s, a in jobs]).astype(np.int32)
            rh = np.stack([tabs[s]["row_hi"][a] for s, a in jobs]).astype(np.int32)
            wlj = np.stack([tabs[s]["wl"][a] for s, a in jobs]).astype(F32)
            whj = np.stack([tabs[s]["wh"][a] for s, a in jobs]).astype(F32)
            return rl, rh, wlj, whj

        rl, rh, wlj, whj = job_rows(ipad)
        irr_idx = np.stack([rl.reshape(-1), rh.reshape(-1)], axis=1)
        Mm = np.stack([tabs[s]["M"][a] for s, a in ipad]).astype(F32)
        irr_w = np.zeros((NIG * P, IRRW), F32)
        irr_w[:, 0] = wlj.reshape(-1)
        irr_w[:, 1] = whj.reshape(-1)
        irr_w[:, 4:] = np.repeat(Mm.reshape(len(ipad), 72), SIZE, axis=0)
        irr_idx_pm = irr_idx.reshape(NIG, P, 2).transpose(1, 0, 2).reshape(P, NIG * 2)
        irr_w_pm = irr_w.reshape(NIG, P, IRRW).transpose(1, 0, 2).reshape(P, NIG * IRRW)

        per_core.append(dict(ridx=np.ascontiguousarray(ridx),
                             wmat=wmat, rbx=rbx,
                             irr_idx=np.ascontiguousarray(irr_idx_pm, np.int32),
                             irr_w=np.ascontiguousarray(irr_w_pm, F32),
                             rjobs=rj, ijobs=ij))

    return dict(cat=cat, per_core=per_core, NRG=NRG, NIG=NIG,
                NPIX=cat.shape[0], A=A)


# ----------------------------------------------------------------------------
# device program
# ----------------------------------------------------------------------------

def _build_program(NPIX, NRG, NIG):
    f32 = mybir.dt.float32
    f16 = mybir.dt.float16
    i32 = mybir.dt.int32
    MULT = mybir.AluOpType.mult
    ADD = mybir.AluOpType.add
    COPY = mybir.ActivationFunctionType.Copy
    OB = 4                       # output groups batched per DMA

    nc = bacc.Bacc("TRN2")
    cat_t = nc.dram_tensor("cat", [NPIX, C], f16, kind="ExternalInput")
    ridx_t = nc.dram_tensor("ridx", [RPART, NRG], i32, kind="ExternalInput")
    wmat_t = nc.dram_tensor("wmat", [RPART, NRG * ROUT], f16, kind="ExternalInput")
    rbx_t = nc.dram_tensor("rbx", [ROUT, NRG * 2], f32, kind="ExternalInput")
    iix_t = nc.dram_tensor("irr_idx", [P, NIG * 2], i32, kind="ExternalInput")
    iw_t = nc.dram_tensor("irr_w", [P, NIG * IRRW], f32, kind="ExternalInput")
    outr_t = nc.dram_tensor("out_reg", [ROUT, NRG * SIZE * C], f32,
                            kind="ExternalOutput")
    outi_t = nc.dram_tensor("out_irr", [P, NIG * SIZE * C], f32,
                            kind="ExternalOutput")

    with TileContext(nc) as tc:
        with tc.tile_pool(name="const", bufs=1) as cpool, \
             tc.tile_pool(name="gat", bufs=8) as gpool, \
             tc.tile_pool(name="ps", bufs=3, space="PSUM") as pspool, \
             tc.tile_pool(name="mid", bufs=6) as upool, \
             tc.tile_pool(name="fin", bufs=5) as opool, \
             tc.tile_pool(name="ob", bufs=3) as obpool:

            ridx = cpool.tile([RPART, NRG], i32)
            wmat = cpool.tile([RPART, NRG * ROUT], f16)
            rbx = cpool.tile([ROUT, NRG * 2], f32)
            iix = cpool.tile([P, NIG * 2], i32)
            iw = cpool.tile([P, NIG * IRRW], f32)
            nc.sync.dma_start(ridx[:, :], ridx_t[:, :])
            nc.sync.dma_start(wmat[:, :], wmat_t[:, :])
            nc.sync.dma_start(rbx[:, :], rbx_t[:, :])
            nc.sync.dma_start(iix[:, :], iix_t[:, :])
            nc.sync.dma_start(iw[:, :], iw_t[:, :])

            # ---------------- REG groups: 9-row gather + PE y-interp
            obuf = None
            ob_base = 0

            def flushr(gi):
                nonlocal obuf, ob_base
                if obuf is not None:
                    nc.sync.dma_start(
                        outr_t[:, ob_base * SIZE * C: gi * SIZE * C],
                        obuf[:, : (gi - ob_base) * SIZE * C])
                    obuf = None

            for g in range(NRG):
                if obuf is None:
                    obuf = obpool.tile([ROUT, OB * SIZE * C], f32, tag="obufr")
                    ob_base = g
                g9 = gpool.tile([RPART, 9 * C], f16, tag="g9")
                nc.gpsimd.indirect_dma_start(
                    out=g9[:, :], out_offset=None, in_=cat_t[:, :],
                    in_offset=bass.IndirectOffsetOnAxis(ap=ridx[:, g:g + 1], axis=0))
                gs = 0
                psA = pspool.tile([ROUT, 9 * C], f32, tag="psA")
                lhsT = wmat[:, ROUT * g: ROUT * (g + 1)]
                nc.tensor.matmul(psA[:, 0:8 * C], lhsT, g9[:, gs: gs + 8 * C],
                                 start=True, stop=True)
                nc.tensor.matmul(psA[:, 8 * C:9 * C], lhsT,
                                 g9[:, gs + 8 * C: gs + 9 * C],
                                 start=True, stop=True)
                m2 = opool.tile([ROUT, SIZE * C], f32, tag="m2")
                nc.scalar.activation(m2[:, :], psA[:, 0:8 * C], COPY,
                                     scale=rbx[:, 2 * g: 2 * g + 1])
                osl = obuf[:, (g - ob_base) * SIZE * C:
                           (g - ob_base + 1) * SIZE * C]
                oslr = osl.rearrange("p (x c) -> p x c", c=C)
                psAr = psA[:, :].rearrange("p (x c) -> p x c", c=C)
                m2r = m2[:, :].rearrange("p (x c) -> p x c", c=C)
                nc.vector.scalar_tensor_tensor(
                    out=oslr, in0=psAr[:, 1:9, :],
                    scalar=rbx[:, 2 * g + 1: 2 * g + 2], in1=m2r,
                    op0=MULT, op1=ADD)
                if g - ob_base + 1 == OB:
                    flushr(g + 1)
            flushr(NRG)

            # ---------------- IRR groups: previous dense path
            def y_stage(idx_ap_lo, idx_ap_hi, wl_ap, wh_ap):
                glo = gpool.tile([P, 9 * C], f16, tag="glo")
                ghi = gpool.tile([P, 9 * C], f16, tag="ghi")
                nc.gpsimd.indirect_dma_start(
                    out=glo[:, :], out_offset=None, in_=cat_t[:, :],
                    in_offset=bass.IndirectOffsetOnAxis(ap=idx_ap_lo, axis=0))
                nc.gpsimd.indirect_dma_start(
                    out=ghi[:, :], out_offset=None, in_=cat_t[:, :],
                    in_offset=bass.IndirectOffsetOnAxis(ap=idx_ap_hi, axis=0))
                m1 = upool.tile([P, 9 * C], f32, tag="m1")
                nc.scalar.activation(m1[:, :], glo[:, :], COPY, scale=wl_ap)
                u = upool.tile([P, 9 * C], f32, tag="u")
                nc.vector.scalar_tensor_tensor(
                    out=u[:, :], in0=ghi[:, :], scalar=wh_ap, in1=m1[:, :],
                    op0=MULT, op1=ADD)
                return u

            obuf = None
            ob_base = 0

            def flushi(gi):
                nonlocal obuf, ob_base
                if obuf is not None:
                    nc.sync.dma_start(
                        outi_t[:, ob_base * SIZE * C: gi * SIZE * C],
                        obuf[:, : (gi - ob_base) * SIZE * C])
                    obuf = None

            for g in range(NIG):
                if obuf is None:
                    obuf = obpool.tile([P, OB * SIZE * C], f32, tag="obufi")
                    ob_base = g
                osl = obuf[:, (g - ob_base) * SIZE * C:
                           (g - ob_base + 1) * SIZE * C]
                u = y_stage(iix[:, 2 * g: 2 * g + 1], iix[:, 2 * g + 1: 2 * g + 2],
                            iw[:, IRRW * g: IRRW * g + 1],
                            iw[:, IRRW * g + 1: IRRW * g + 2])
                ur = u[:, :].rearrange("p (x c) -> p x c", c=C)
                acc0 = opool.tile([P, SIZE * C], f32, tag="acc0")
                acc1 = opool.tile([P, SIZE * C], f32, tag="acc1")
                acc = [acc0, acc1]
                tmp = opool.tile([P, SIZE * C], f32, tag="tmp")
                wbase = IRRW * g + 4
                for e in range(9):
                    u_e = ur[:, e:e + 1, :].to_broadcast([P, SIZE, C])
                    m_e = (iw[:, wbase + e * SIZE: wbase + (e + 1) * SIZE]
                           .rearrange("p (j u) -> p j u", u=1)
                           .to_broadcast([P, SIZE, C]))
                    dst = acc[0] if e == 0 else tmp
                    dstr = dst[:, :].rearrange("p (j c) -> p j c", c=C)
                    nc.vector.tensor_tensor(out=dstr, in0=u_e, in1=m_e, op=MULT)
                    if 0 < e < 8:
                        nc.vector.tensor_tensor(out=acc[e % 2][:, :],
                                                in0=acc[(e - 1) % 2][:, :],
                                                in1=tmp[:, :], op=ADD)
                    elif e == 8:
                        nc.vector.tensor_tensor(out=osl, in0=acc[1][:, :],
                                                in1=tmp[:, :], op=ADD)
                if g - ob_base + 1 == OB:
                    flushi(g + 1)
            flushi(NIG)

    nc.finalize()
    return nc


# ----------------------------------------------------------------------------
# entry point
# ----------------------------------------------------------------------------

def kernel(f0, f1, f2, pixel, batch_index):
    global LAST_RESULTS
    prep = _host_prep(f0, f1, f2, pixel, batch_index)
    NRG, NIG, A = prep["NRG"], prep["NIG"], prep["A"]

    nc = _build_program(prep["NPIX"], NRG, NIG)

    in_maps = []
    for ccc in range(N_CORES):
        pc = prep["per_core"][ccc]
        in_maps.append({"cat": prep["cat"], "ridx": pc["ridx"],
                        "wmat": pc["wmat"], "rbx": pc["rbx"],
                        "irr_idx": pc["irr_idx"], "irr_w": pc["irr_w"]})

    res = run_bass_kernel_spmd(nc, in_maps, core_ids=list(range(N_CORES)),
                               trace=bool(os.environ.get("BASS_TRACE")))
    LAST_RESULTS = res

    out = np.zeros((A, 3, C, SIZE, SIZE), F32)
    for ccc in range(N_CORES):
        pc = prep["per_core"][ccc]
        rj, ij = pc["rjobs"], pc["ijobs"]
        # reg: [112, NRG*512] -> [NRG*14, i, jx, c]
        raw = res.results[ccc]["out_reg"].reshape(ROUT, NRG, SIZE, C)
        dev = (raw.transpose(1, 0, 2, 3)
               .reshape(NRG, RJOBS, SIZE, SIZE, C)
               .reshape(-1, SIZE, SIZE, C))
        if len(rj):
            out[rj[:, 1], rj[:, 0]] = dev[:len(rj)].transpose(0, 3, 1, 2)
        # irr: [128, NIG*512] -> [NIG*16, i, jx, c]
        raw = res.results[ccc]["out_irr"].reshape(P, NIG, SIZE * C)
        dev = (raw.transpose(1, 0, 2)
               .reshape(NIG, IJOBS, SIZE, SIZE, C)
               .reshape(-1, SIZE, SIZE, C))
        if len(ij):
            out[ij[:, 1], ij[:, 0]] = dev[:len(ij)].transpose(0, 3, 1, 2)
    return out.reshape(A, 3 * C, SIZE, SIZE)
